# revision 2
# baseline (speedup 1.0000x reference)
"""BitLinear (int8-activation x ternary-weight) matmul on 8 TRN2 NeuronCores.

Full inputs: x [4, 4096, 2048] f32, weight [2048, 2048] f32.
Output: [4, 4096, 2048] fp16 = ((qx @ qw.T) / si / sw).astype(f16).

Data-parallel over the 16384 rows (2048 rows/core). The serial head of
the previous version (full 16.8MB W read -> mean|W| -> quantize ->
first matmul at ~84us) is broken up three ways:

1. Distributed mean|W|: each core contributes the |w|-sums of 2 of the
   16 W k-tiles (its first two in a per-core rotated k-order, prepared
   host-side as a pure block permutation of BOTH wt's k-rows and x's
   k-columns so the dot products are order-invariant). A [128,1] f32
   AllReduce gives every core the exact global sum ~40us earlier than
   a local full read would.
2. Quantize-on-arrival: once sw is known (~15-20us), W k-tiles are
   ternarized as they stream in; no second W read, no big raw cache.
3. DMA ring split: the sync HWDGE ring carries ONLY the input loads
   (W tiles first, then x), so the critical W stream is never
   head-of-line blocked; qx transposes and output stores ride the
   scalar (ACT) HWDGE ring.

Row tiles 0,1 interleave their matmuls across arriving W k-tiles
(accumulation order per PSUM group follows the local k order). Rows
2-15 then run back-to-back at the PE streaming rate.

Matmul precision: activations are quantized to int8 values held in
bf16 via the fp32 magic-number trick; ternary weights are held in fp8.
K-tiles 2..7 (per-core local order) additionally run as fp8e4 DoubleRow
pairs: the bf16 integer activations are RNE-cast to fp8e4 (a deliberate
~1.7e-2 rel-err approximation on 6/16 of the contraction, inside the
2e-2 gate) and each DoubleRow matmul covers 256 contraction elements
per 512-cycle pass, trimming PE time. The dequant (acc * amax/127 *
mean|W|) is fused into the PSUM->SBUF fp16 copy on the ScalarEngine.
"""

import numpy as np

import concourse.mybir as mybir
import concourse.tile as tile
from concourse import bacc
from concourse.bass import ts
from concourse.bass_utils import run_bass_kernel_spmd

N_CORES = 8
ROWS_TOTAL = 4 * 4096
K = 2048
N = 2048
MAGIC = 12582912.0  # 1.5*2^23: fp32 round-to-nearest-even (both signs)

KT = K // 128  # 16 k-tiles
NQ = N // 512  # 4 psum-width output chunks
NPAIRS = 3  # fp8 DoubleRow pairs: local k-tiles [2, 2+2*NPAIRS)
PAIR_LO = 2
PAIR_HI = PAIR_LO + 2 * NPAIRS

f32 = mybir.dt.float32
bf16 = mybir.dt.bfloat16
f16 = mybir.dt.float16
fp8 = mybir.dt.float8e4
Alu = mybir.AluOpType
Act = mybir.ActivationFunctionType
AxX = mybir.AxisListType.X
DR = mybir.MatmulPerfMode.DoubleRow


def build(rows_per_core=ROWS_TOTAL // N_CORES):
    nc = bacc.Bacc(
        "TRN2", target_bir_lowering=False, debug=False, num_devices=N_CORES
    )
    x_ext = nc.declare_dram_parameter("x", [rows_per_core, K], f32, isOutput=False)
    wt_ext = nc.declare_dram_parameter("wt", [K, N], f32, isOutput=False)
    out_ext = nc.declare_dram_parameter(
        "out", [rows_per_core, N], f16, isOutput=True
    )

    MT = rows_per_core // 128

    def is_pair_lead(kt):
        return NPAIRS and PAIR_LO <= kt < PAIR_HI and (kt - PAIR_LO) % 2 == 0

    def in_pair(kt):
        return NPAIRS and PAIR_LO <= kt < PAIR_HI

    with tile.TileContext(nc) as tc:
        with (
            tc.tile_pool(name="xin", bufs=5) as xin,  # [128,K] f32 x loads
            tc.tile_pool(name="wld", bufs=9) as wld,  # [128,K] f32 W loads
            tc.tile_pool(name="scaled", bufs=2) as scaled,  # x*si+MAGIC f32
            tc.tile_pool(name="qtmp", bufs=2) as qtmp,  # qx bf16
            tc.tile_pool(name="qxt", bufs=3) as qxtp,  # [128,KT,128] bf16 x^T
            tc.tile_pool(name="qx8", bufs=3) as qx8p,  # fp8 x^T pair slices
            tc.tile_pool(name="outp", bufs=3) as outp,  # [128,N] f16 results
            tc.tile_pool(name="singles", bufs=1) as singles,
            tc.tile_pool(name="small", bufs=8) as small,  # [128,1] stats
            tc.tile_pool(name="pacc", bufs=8, space="PSUM") as pacc,
            tc.tile_pool(name="dram", bufs=2, space="DRAM") as dram,
        ):
            ones_mat = singles.tile([128, 128], f32)
            nc.vector.memset(ones_mat, 1.0)
            negmagic_b = singles.tile([128, 1], f32)
            nc.vector.memset(negmagic_b, -MAGIC)
            wsums = singles.tile([128, 2], f32)
            # quantized W storage: pair-layout tiles for the DoubleRow
            # k-tiles, plain tiles for the rest (separate tensors so a
            # matmul's dependency is exactly one Sign write)
            qw_sing = {}
            qw_pair = {}
            for kt in range(KT):
                if is_pair_lead(kt):
                    j = (kt - PAIR_LO) // 2
                    qw_pair[j] = singles.tile([128, 2, N], fp8, name=f"qwp{j}")
                elif not in_pair(kt):
                    qw_sing[kt] = singles.tile([128, N], fp8, name=f"qws{kt}")

            def qw_slot(kt):
                if in_pair(kt):
                    j = (kt - PAIR_LO) // 2
                    return qw_pair[j][:, (kt - PAIR_LO) % 2, :]
                return qw_sing[kt][:, :]

            # ---- sync ring: W tiles 0,1 then x 0..3 then W 2..15
            w_tiles = {}
            for kt in range(2):
                wt_t = wld.tile([128, K], f32, tag="wld", name=f"w{kt}")
                nc.sync.dma_start(out=wt_t, in_=wt_ext[ts(kt, 128), :])
                w_tiles[kt] = wt_t
            x_tiles = {}
            for mi in range(4):
                x_t = xin.tile([128, K], f32, tag="xin", name=f"x{mi}")
                nc.sync.dma_start(out=x_t, in_=x_ext[ts(mi, 128), :])
                x_tiles[mi] = x_t
            for kt in range(2, KT):
                wt_t = wld.tile([128, K], f32, tag="wld", name=f"w{kt}")
                nc.sync.dma_start(out=wt_t, in_=wt_ext[ts(kt, 128), :])
                w_tiles[kt] = wt_t

            # ---- distributed mean|W|: local |w|-sums of tiles 0,1 -> AR
            for kt in range(2):
                nc.vector.tensor_reduce(
                    out=wsums[:, kt : kt + 1], in_=w_tiles[kt], axis=AxX,
                    op=Alu.add, apply_absolute_value=True,
                )
            w2 = small.tile([128, 1], f32, tag="w2")
            nc.vector.tensor_reduce(out=w2, in_=wsums, axis=AxX, op=Alu.add)
            ar_in = dram.tile([128, 1], f32)
            ar_out = dram.tile([128, 1], f32)
            nc.gpsimd.dma_start(ar_in[:], w2)
            nc.gpsimd.collective_compute(
                "AllReduce", Alu.add,
                replica_groups=[list(range(N_CORES))],
                ins=[ar_in.opt()], outs=[ar_out.opt()],
            )
            s_t = small.tile([128, 1], f32, tag="s_t")
            nc.gpsimd.dma_start(s_t, ar_out[:])

            # ---- x-quant helpers (DVE scale pass, ACT unmagic pass)
            def xq_dve(mi):
                x_t = x_tiles[mi]
                amax = small.tile([128, 1], f32, tag="small")
                nc.vector.tensor_reduce(
                    out=amax, in_=x_t, axis=AxX, op=Alu.max,
                    apply_absolute_value=True,
                )
                amc = small.tile([128, 1], f32, tag="amc", name=f"amc{mi}")
                nc.vector.tensor_scalar_max(out=amc, in0=amax, scalar1=1e-5)
                rec = small.tile([128, 1], f32, tag="small")
                nc.vector.reciprocal(out=rec, in_=amc)
                si = small.tile([128, 1], f32, tag="small")
                nc.vector.tensor_scalar_mul(out=si, in0=rec, scalar1=127.0)
                xs = scaled.tile([128, K], f32, tag="scaled")
                nc.vector.tensor_scalar(
                    out=xs, in0=x_t, scalar1=si, scalar2=MAGIC,
                    op0=Alu.mult, op1=Alu.add,
                )
                return xs, amc

            def xq_act(mi, xs):
                qx = qtmp.tile([128, K], bf16, tag="qtmp")
                nc.scalar.activation(out=qx, in_=xs, func=Act.Copy, bias=-MAGIC)
                qxT = qxtp.tile([128, KT, 128], bf16, tag="qxt", name=f"qxT{mi}")
                nc.scalar.dma_start_transpose(out=qxT, in_=qx)
                if NPAIRS:
                    qxT8 = qx8p.tile(
                        [128, 2 * NPAIRS, 128], fp8, tag="qx8", name=f"qx8{mi}"
                    )
                    nc.vector.tensor_scalar_mul(
                        out=qxT8, in0=qxT[:, PAIR_LO:PAIR_HI, :], scalar1=1.0
                    )
                else:
                    qxT8 = None
                return qxT, qxT8

            # rows 0,1 quant: DVE passes first, ACT passes next
            xs0, amc0 = xq_dve(0)
            xs1, amc1 = xq_dve(1)
            qxT0, qx80 = xq_act(0, xs0)
            qxT1, qx81 = xq_act(1, xs1)

            # ---- sw from the AllReduce total
            ptot_b = pacc.tile([128, 1], f32, tag="acc", name="ptot_b")
            nc.tensor.matmul(ptot_b, lhsT=ones_mat, rhs=s_t, start=True, stop=True)
            meanc_b = small.tile([128, 1], f32, tag="s1")
            nc.vector.tensor_scalar(
                out=meanc_b, in0=ptot_b, scalar1=1.0 / (K * N), scalar2=1e-5,
                op0=Alu.mult, op1=Alu.max,
            )
            sw_b = singles.tile([128, 1], f32)
            nc.vector.reciprocal(out=sw_b, in_=meanc_b)
            q_b = singles.tile([128, 1], f32)
            nc.vector.tensor_scalar_mul(out=q_b, in0=meanc_b, scalar1=1.0 / 127.0)

            # ---- W pass2 on arrival: u = w*sw + MAGIC (DVE, in-place);
            # Sign(u - MAGIC) -> fp8 (ACT). x2/x3 quant interleaved so
            # their engine-queue slots sit at the right readiness times.
            def wq(kt):
                wt_t = w_tiles[kt]
                nc.vector.tensor_scalar(
                    out=wt_t, in0=wt_t, scalar1=sw_b, scalar2=MAGIC,
                    op0=Alu.mult, op1=Alu.add,
                )
                nc.scalar.activation(
                    out=qw_slot(kt), in_=wt_t, func=Act.Sign, bias=negmagic_b
                )

            for kt in range(6):
                wq(kt)
            xs2, amc2 = xq_dve(2)
            qxT2, qx82 = xq_act(2, xs2)
            for kt in range(6, 9):
                wq(kt)
            xs3, amc3 = xq_dve(3)
            qxT3, qx83 = xq_act(3, xs3)
            for kt in range(9, KT):
                wq(kt)

            # ---- matmul emission per row tile: local-k order, DoubleRow
            # pairs covering k-tiles [PAIR_LO, PAIR_HI)
            def row_mms(accs, qxT, qxT8, nq_inner=False):
                steps = []  # (kind, idx)
                kt = 0
                while kt < KT:
                    if is_pair_lead(kt):
                        steps.append(("pair", (kt - PAIR_LO) // 2))
                        kt += 2
                    else:
                        steps.append(("sing", kt))
                        kt += 1

                def emit(step_i, nq):
                    kind, idx = steps[step_i]
                    first = step_i == 0
                    last = step_i == len(steps) - 1
                    if kind == "sing":
                        nc.tensor.matmul(
                            accs[nq], lhsT=qxT[:, idx, :],
                            rhs=qw_sing[idx][:, ts(nq, 512)],
                            start=first, stop=last, skip_group_check=True,
                        )
                    else:
                        lo = 2 * idx
                        nc.tensor.matmul(
                            accs[nq], lhsT=qxT8[:, lo : lo + 2, :],
                            rhs=qw_pair[idx][:, :, ts(nq, 512)],
                            start=first, stop=last, skip_group_check=True,
                            perf_mode=DR,
                        )

                if nq_inner:
                    for nq in range(NQ):
                        for si_ in range(len(steps)):
                            emit(si_, nq)
                else:
                    for si_ in range(len(steps)):
                        for nq in range(NQ):
                            emit(si_, nq)

            def finish(mi, accs, amc, chunked=False):
                cs = small.tile([128, 1], f32, tag="small")
                nc.vector.tensor_mul(cs, amc, q_b)  # (amax/127)*meanc
                o_t = outp.tile([128, N], f16, tag="outp", name=f"o{mi}")
                for nq in range(NQ):
                    nc.scalar.activation(
                        out=o_t[:, ts(nq, 512)], in_=accs[nq],
                        func=Act.Copy, scale=cs,
                    )
                    if chunked:
                        nc.scalar.dma_start(
                            out=out_ext[ts(mi, 128), ts(nq, 512)],
                            in_=o_t[:, ts(nq, 512)],
                        )
                if not chunked:
                    nc.scalar.dma_start(out=out_ext[ts(mi, 128), :], in_=o_t)

            # ---- ramp: rows 0,1 interleaved across local k arrival order
            def mk_accs(mi):
                return [
                    pacc.tile([128, 512], f32, tag="acc", name=f"acc_{mi}_{i}")
                    for i in range(NQ)
                ]

            steps_n = KT - NPAIRS  # mm steps per (row, nq)
            accs0 = mk_accs(0)
            accs1 = mk_accs(1)
            ramp = [(accs0, qxT0, qx80), (accs1, qxT1, qx81)]
            kt = 0
            step_i = 0
            while kt < KT:
                lead = is_pair_lead(kt)
                for accs, qxT, qxT8 in ramp:
                    first = step_i == 0
                    last = step_i == steps_n - 1
                    for nq in range(NQ):
                        if lead:
                            lo = kt - PAIR_LO
                            nc.tensor.matmul(
                                accs[nq], lhsT=qxT8[:, lo : lo + 2, :],
                                rhs=qw_pair[lo // 2][:, :, ts(nq, 512)],
                                start=first, stop=last, skip_group_check=True,
                                perf_mode=DR,
                            )
                        else:
                            nc.tensor.matmul(
                                accs[nq], lhsT=qxT[:, kt, :],
                                rhs=qw_sing[kt][:, ts(nq, 512)],
                                start=first, stop=last, skip_group_check=True,
                            )
                kt += 2 if lead else 1
                step_i += 1
            finish(0, accs0, amc0)
            finish(1, accs1, amc1)

            # ---- steady rows
            row_in = {2: (qxT2, qx82, amc2), 3: (qxT3, qx83, amc3)}
            for mi in range(2, MT):
                if mi in row_in:
                    qxT, qxT8, amc = row_in[mi]
                else:
                    x_t = xin.tile([128, K], f32, tag="xin", name=f"x{mi}")
                    nc.sync.dma_start(out=x_t, in_=x_ext[ts(mi, 128), :])
                    x_tiles[mi] = x_t
                    xs, amc = xq_dve(mi)
                    qxT, qxT8 = xq_act(mi, xs)
                accs = mk_accs(mi)
                row_mms(accs, qxT, qxT8, nq_inner=(mi == MT - 1))
                finish(mi, accs, amc, chunked=(mi == MT - 1))

    nc.compile()
    return nc


_NC_CACHE = {}


def _get_nc(rows_per_core):
    if rows_per_core not in _NC_CACHE:
        _NC_CACHE[rows_per_core] = build(rows_per_core)
    return _NC_CACHE[rows_per_core]


def run(x, weight, **spmd_kwargs):
    x = np.ascontiguousarray(np.asarray(x, dtype=np.float32))
    weight = np.asarray(weight, dtype=np.float32)
    b, s, k = x.shape
    rows = b * s
    rpc = rows // N_CORES
    xr = x.reshape(rows, k)
    wt = np.ascontiguousarray(weight.T)  # [K, N]
    nc = _get_nc(rpc)
    kt = k // 128
    wt_blocks = wt.reshape(kt, 128, N)
    in_maps = []
    for c in range(N_CORES):
        perm = [(2 * c + j) % kt for j in range(kt)]
        wt_c = np.ascontiguousarray(wt_blocks[perm].reshape(k, N))
        xs_c = xr[c * rpc : (c + 1) * rpc]
        x_c = np.ascontiguousarray(
            xs_c.reshape(rpc, kt, 128)[:, perm, :].reshape(rpc, k)
        )
        in_maps.append({"x": x_c, "wt": wt_c})
    res = run_bass_kernel_spmd(
        nc, in_maps, core_ids=list(range(N_CORES)), **spmd_kwargs
    )
    out = np.concatenate(
        [res.results[i]["out"] for i in range(N_CORES)], axis=0
    )
    return out.reshape(b, s, N), res


def kernel(x, weight):
    out, _ = run(x, weight)
    return out


# revision 6
# speedup vs baseline: 1.0843x; 1.0843x over previous
"""BitLinear (int8-activation x ternary-weight) matmul on 8 TRN2 NeuronCores.

Full inputs: x [4, 4096, 2048] f32, weight [2048, 2048] f32.
Output: [4, 4096, 2048] fp16 = ((qx @ qw.T) / si / sw).astype(f16).

Data-parallel over the 16384 rows (2048 rows/core). The weight is
replicated; each core computes mean|W| on-device during the single
streaming W read, quantizes W to ternary {-1,0,1} stored as fp8, and
runs bf16(lhsT=qx^T) x fp8(qw^T) matmuls with fp32 PSUM accumulation
(exact for these integer values). Dequant (acc * amax/127 * mean|W|)
is fused into the PSUM->SBUF fp16 copy on the ScalarEngine.

Head structure (the serial part before the first matmul):
- The sync HWDGE ring carries ONLY input loads, ordered W0,W1,x0,
  W2..W15 — so mean|W| (and thus sw) lands right as the W stream ends,
  with x0's quantized transpose already waiting. DMA-xbar transposes
  are emitted AFTER the W stream (HWDGE FIFO + the transpose
  serialization guard would otherwise head-of-line block the loads).
- 12 raw W k-tiles stay cached in SBUF; the 4 that could not be held
  (12..15) are re-read once sw is known, through 2 rotating bounce
  bufs so their first read (needed for the mean) is never blocked on
  a buf that only frees after sw (which would deadlock).
- Rows 0,1 interleave their matmuls across the arriving ternarized
  k-tiles; rows 2..15 then run back-to-back at the PE streaming rate
  (~216ns per [128x128]x[128x512] bf16 matmul).

Activation quantization: DVE computes per-row amax and x*si + 1.5*2^23
(the fp32 magic-number add rounds to nearest-even); ACT subtracts the
magic into bf16. fp8 DoubleRow was tried and measured: it trips the
board GPIO power throttle (PE clamped to 13/16 clock) which cancels
the double-pump gain, so everything stays bf16/fp8-moving and exact.
"""

import numpy as np

import concourse.mybir as mybir
import concourse.tile as tile
from concourse import bacc
from concourse.bass import ts
from concourse.bass_utils import run_bass_kernel_spmd

N_CORES = 8
ROWS_TOTAL = 4 * 4096
K = 2048
N = 2048
MAGIC = 12582912.0  # 1.5*2^23: fp32 round-to-nearest-even (both signs)

KT = K // 128  # 16 k-tiles
NQ = N // 512  # 4 psum-width output chunks
WCACHE = 11  # raw W k-tiles held in SBUF; tiles WCACHE..15 are re-read

f32 = mybir.dt.float32
bf16 = mybir.dt.bfloat16
f16 = mybir.dt.float16
fp8 = mybir.dt.float8e4
Alu = mybir.AluOpType
Act = mybir.ActivationFunctionType
AxX = mybir.AxisListType.X


def build(rows_per_core=ROWS_TOTAL // N_CORES):
    nc = bacc.Bacc(
        "TRN2", target_bir_lowering=False, debug=False, num_devices=N_CORES
    )
    x_ext = nc.declare_dram_parameter("x", [rows_per_core, K], f32, isOutput=False)
    wt_ext = nc.declare_dram_parameter("wt", [K, N], f32, isOutput=False)
    out_ext = nc.declare_dram_parameter(
        "out", [rows_per_core, N], f16, isOutput=True
    )

    MT = rows_per_core // 128

    with tile.TileContext(nc) as tc:
        with (
            tc.tile_pool(name="xin", bufs=4) as xin,  # [128,K] f32 x loads
            tc.tile_pool(name="wld", bufs=WCACHE) as wld,  # cached raw W
            tc.tile_pool(name="wtmp", bufs=2) as wtmp,  # W 12..15 bounce
            tc.tile_pool(name="scaled", bufs=1) as scaled,  # x*si+MAGIC f32
            tc.tile_pool(name="qtmp", bufs=2) as qtmp,  # qx bf16
            tc.tile_pool(name="qxt", bufs=3) as qxtp,  # [128,KT,128] bf16 x^T
            tc.tile_pool(name="outp", bufs=2) as outp,  # [128,N] f16 results
            tc.tile_pool(name="singles", bufs=1) as singles,
            tc.tile_pool(name="small", bufs=8) as small,  # [128,1] stats
            tc.tile_pool(name="pacc", bufs=8, space="PSUM") as pacc,
        ):
            ones_mat = singles.tile([128, 128], f32)
            nc.vector.memset(ones_mat, 1.0)
            negmagic_b = singles.tile([128, 1], f32)
            nc.vector.memset(negmagic_b, -MAGIC)
            wsums = singles.tile([128, KT], f32)
            qwT = singles.tile([128, KT, N], fp8)

            # ---- sync ring: W0, W1, x0, W2..W15 (nothing else may sit
            # ahead of the W stream)
            w_tiles = {}

            def w_load(kt, pool, tag):
                wt_t = pool.tile([128, K], f32, tag=tag, name=f"w{kt}")
                nc.sync.dma_start(out=wt_t, in_=wt_ext[ts(kt, 128), :])
                w_tiles[kt] = wt_t
                return wt_t

            x_tiles = {}

            def x_load(mi):
                x_t = xin.tile([128, K], f32, tag="xin", name=f"x{mi}")
                nc.sync.dma_start(out=x_t, in_=x_ext[ts(mi, 128), :])
                x_tiles[mi] = x_t

            w_load(0, wld, "wld")
            w_load(1, wld, "wld")
            x_load(0)
            for kt in range(2, WCACHE):
                w_load(kt, wld, "wld")
            for kt in range(WCACHE, KT):
                w_load(kt, wtmp, "wtmp")

            # ---- x-quant pieces (emission split so each engine queue
            # sees its ops in readiness order)
            amcs = {}

            def xq_dve(mi):
                x_t = x_tiles[mi]
                amax = small.tile([128, 1], f32, tag="small")
                nc.vector.tensor_reduce(
                    out=amax, in_=x_t, axis=AxX, op=Alu.max,
                    apply_absolute_value=True,
                )
                amc = small.tile([128, 1], f32, tag="amc", name=f"amc{mi}")
                nc.vector.tensor_scalar_max(out=amc, in0=amax, scalar1=1e-5)
                rec = small.tile([128, 1], f32, tag="small")
                nc.vector.reciprocal(out=rec, in_=amc)
                si = small.tile([128, 1], f32, tag="small")
                nc.vector.tensor_scalar_mul(out=si, in0=rec, scalar1=127.0)
                xs = scaled.tile([128, K], f32, tag="scaled")
                nc.vector.tensor_scalar(
                    out=xs, in0=x_t, scalar1=si, scalar2=MAGIC,
                    op0=Alu.mult, op1=Alu.add,
                )
                amcs[mi] = amc
                return xs

            qxs = {}

            def xq_act(mi, xs):
                qx = qtmp.tile([128, K], bf16, tag="qtmp")
                nc.scalar.activation(out=qx, in_=xs, func=Act.Copy, bias=-MAGIC)
                qxs[mi] = qx

            qxTs = {}

            def xq_transpose(mi):
                qxT = qxtp.tile([128, KT, 128], bf16, tag="qxt", name=f"qxT{mi}")
                nc.sync.dma_start_transpose(out=qxT, in_=qxs.pop(mi))
                qxTs[mi] = qxT

            # DVE/ACT for x0 queued ahead of the |w| sums so they run on
            # x0's arrival while the W stream continues
            xq_act(0, xq_dve(0))

            # ---- mean|W| during the stream
            for kt in range(KT):
                nc.vector.tensor_reduce(
                    out=wsums[:, kt : kt + 1], in_=w_tiles[kt], axis=AxX,
                    op=Alu.add, apply_absolute_value=True,
                )
            wtot = small.tile([128, 1], f32, tag="small")
            nc.vector.tensor_reduce(out=wtot, in_=wsums, axis=AxX, op=Alu.add)
            # ones_mat.T @ wtot replicates the grand total across all 128
            # partitions so the scale math stays [128,1]
            ptot_b = pacc.tile([128, 1], f32, tag="acc", name="ptot_b")
            nc.tensor.matmul(ptot_b, lhsT=ones_mat, rhs=wtot, start=True, stop=True)
            meanc_b = small.tile([128, 1], f32, tag="s1")
            nc.vector.tensor_scalar(
                out=meanc_b, in0=ptot_b, scalar1=1.0 / (K * N), scalar2=1e-5,
                op0=Alu.mult, op1=Alu.max,
            )
            sw_b = singles.tile([128, 1], f32)
            nc.vector.reciprocal(out=sw_b, in_=meanc_b)
            q_b = singles.tile([128, 1], f32)
            nc.vector.tensor_scalar_mul(out=q_b, in0=meanc_b, scalar1=1.0 / 127.0)

            # ---- W pass2: u = w*sw + MAGIC (DVE in-place), then
            # Sign(u - MAGIC) -> fp8 (ACT); for integer n, sign(n) ==
            # clip(n, -1, 1)
            def wq(kt):
                wt_t = w_tiles[kt]
                nc.vector.tensor_scalar(
                    out=wt_t, in0=wt_t, scalar1=sw_b, scalar2=MAGIC,
                    op0=Alu.mult, op1=Alu.add,
                )
                nc.scalar.activation(
                    out=qwT[:, kt, :], in_=wt_t, func=Act.Sign, bias=negmagic_b
                )

            # sync ring after the W stream: x0's transpose first (it gates
            # the first matmul), then x1 + its transpose, then the W12..15
            # re-reads through the wtmp rotation. Engine queues are fed in
            # readiness order: Sign0/Sign1 ahead of x1's quant chain so the
            # ramp starts the moment sw lands.
            xq_transpose(0)
            x_load(1)
            wq(0)
            wq(1)
            xq_act(1, xq_dve(1))
            xq_transpose(1)
            for kt in range(2, WCACHE):
                wq(kt)
            for kt in range(WCACHE, KT):
                w_load(kt, wtmp, "wtmp")
                wq(kt)

            # ---- matmuls
            def mm(acc, qxT, kt, nq):
                nc.tensor.matmul(
                    acc, lhsT=qxT[:, kt, :], rhs=qwT[:, kt, ts(nq, 512)],
                    start=(kt == 0), stop=(kt == KT - 1),
                    skip_group_check=True,
                )

            def mk_accs(mi):
                return [
                    pacc.tile([128, 512], f32, tag="acc", name=f"acc_{mi}_{i}")
                    for i in range(NQ)
                ]

            def finish(mi, accs, chunked=False):
                cs = small.tile([128, 1], f32, tag="small")
                nc.vector.tensor_mul(cs, amcs.pop(mi), q_b)  # (amax/127)*meanc
                o_t = outp.tile([128, N], f16, tag="outp", name=f"o{mi}")
                for nq in range(NQ):
                    nc.scalar.activation(
                        out=o_t[:, ts(nq, 512)], in_=accs[nq],
                        func=Act.Copy, scale=cs,
                    )
                    if chunked:
                        nc.scalar.dma_start(
                            out=out_ext[ts(mi, 128), ts(nq, 512)],
                            in_=o_t[:, ts(nq, 512)],
                        )
                if not chunked:
                    nc.scalar.dma_start(out=out_ext[ts(mi, 128), :], in_=o_t)

            # ramp: rows 0,1 interleaved per k-tile so each arriving
            # ternarized k-tile unlocks matmuls; row 1 trails by 4 k-tiles
            # so its first op (gated on qxT1) never head-of-line blocks
            # row 0's stream in the PE queue
            LAG = 4
            accs0 = mk_accs(0)
            accs1 = mk_accs(1)
            for step in range(KT + LAG):
                if step < KT:
                    for nq in range(NQ):
                        mm(accs0[nq], qxTs[0], step, nq)
                if step >= LAG:
                    for nq in range(NQ):
                        mm(accs1[nq], qxTs[1], step - LAG, nq)
            finish(0, accs0)
            finish(1, accs1)
            del qxTs[0], qxTs[1]

            # steady rows
            for mi in range(2, MT):
                x_load(mi)
                xq_act(mi, xq_dve(mi))
                xq_transpose(mi)
                qxT = qxTs.pop(mi)
                accs = mk_accs(mi)
                if mi == MT - 1:
                    # nq-inner: each output chunk completes as soon as its
                    # 16 accumulations are done (shorter kernel tail)
                    for nq in range(NQ):
                        for kt in range(KT):
                            mm(accs[nq], qxT, kt, nq)
                else:
                    for kt in range(KT):
                        for nq in range(NQ):
                            mm(accs[nq], qxT, kt, nq)
                finish(mi, accs, chunked=(mi == MT - 1))

    nc.compile()
    return nc


_NC_CACHE = {}


def _get_nc(rows_per_core):
    if rows_per_core not in _NC_CACHE:
        _NC_CACHE[rows_per_core] = build(rows_per_core)
    return _NC_CACHE[rows_per_core]


def run(x, weight, **spmd_kwargs):
    x = np.ascontiguousarray(np.asarray(x, dtype=np.float32))
    weight = np.asarray(weight, dtype=np.float32)
    b, s, k = x.shape
    rows = b * s
    rpc = rows // N_CORES
    xr = x.reshape(rows, k)
    wt = np.ascontiguousarray(weight.T)
    nc = _get_nc(rpc)
    in_maps = [
        {"x": xr[i * rpc : (i + 1) * rpc], "wt": wt} for i in range(N_CORES)
    ]
    res = run_bass_kernel_spmd(
        nc, in_maps, core_ids=list(range(N_CORES)), **spmd_kwargs
    )
    out = np.concatenate(
        [res.results[i]["out"] for i in range(N_CORES)], axis=0
    )
    return out.reshape(b, s, N), res


def kernel(x, weight):
    out, _ = run(x, weight)
    return out
